# revision 13
# baseline (speedup 1.0000x reference)
"""Trainium2 Bass kernel for nn_ContrastLoss (band-limited PSD contrastive loss).

Math notes (all exact identities, not approximations):
  - reference subtracts the per-window mean, but integer-frequency DFT bins
    23..102 are orthogonal to DC, so mean subtraction is a no-op on the band.
  - the band PSD is normalized per window (band / band.sum()), so the
    reference's 1/DELTA_T rfft scaling cancels; raw |X_f|^2 suffices.
  - rfft band bins are two real matmuls: X_f = seg @ cos_f, seg @ sin_f.

Device work per core (8-way channel sharding, 1024 windows/core):
  Inputs ship as fp8 e4m3 (validated: scale-rel err ~9e-5 vs 2e-2 gate).
  seg stream = 1MB in 5 DMAs (c0, c1 single chunks for a fast start, 2-chunk
  groups after) across both DGE rings; basis w = 160KB.  Per 128-window
  chunk: 4 DoubleRow fp8 matmuls (contraction 256 each) -> its own PSUM bank
  [128,160] f32; ACT squares PSUM->SBUF; Pool folds cos^2+sin^2 -> [128,80]
  bf16 slices of one out tile; one 160KB writeback.

Synchronization is hand-rolled (no TileContext): 11 semaphores, chunk-level
producer counters per engine pair.  The TileContext teardown (one sem-clear
instruction per allocated sem per engine, ~50 sems x ~115ns) was ~8us of the
baseline exec time; here the epilogue is two barriers + one range-clear.
Host: window gather + fp8 shard prep; row-sums, normalization and the
closed-form pairwise-MSE scalars in float64 (cheap: 8x1024x80 values).
"""

import sys

import numpy as np

if "/opt/trn_rl_repo" not in sys.path:
    sys.path.insert(0, "/opt/trn_rl_repo")

import ml_dtypes

B = 2
C = 256
T = 8192
K = 16
DT = 1024
NCORES = 8
CLOC = C // NCORES          # channels per core
SEGS = B * CLOC * K         # windows per core = 1024
F_LO, F_HI = 23, 103        # band bins [23, 102]
NF = F_HI - F_LO            # 80
NW = 2 * NF                 # 160 (cos || sin)
MCH = SEGS // 128           # 8 window chunks
KCH = DT // 128             # 8 contraction chunks
N_TOT = C * K               # 4096 windows per video
GROUPS = ((0,), (1,), (2, 3), (4, 5), (6, 7))   # seg DMA batching


def _dft_basis():
    t = np.arange(DT, dtype=np.float64)
    f = np.arange(F_LO, F_HI, dtype=np.float64)
    ang = 2.0 * np.pi * np.outer(t, f) / DT
    w = np.concatenate([np.cos(ang), np.sin(ang)], axis=1).astype(np.float32)
    w8 = w.astype(ml_dtypes.float8_e4m3fn)      # [DT, NW]
    # [p, k, j] = W[k*128 + p, j] so rhs for k-pair is a dim-1 slice
    return np.ascontiguousarray(
        w8.reshape(KCH, 128, NW).transpose(1, 0, 2)
    )


_W_F8 = _dft_basis()
_NC = None


def _build_nc():
    from contextlib import ExitStack

    import concourse.mybir as mybir
    from concourse import bacc

    nc = bacc.Bacc(
        "TRN2",
        target_bir_lowering=False,
        debug=False,
        enable_asserts=True,
        num_devices=NCORES,
    )
    f32 = mybir.dt.float32
    bf16 = mybir.dt.bfloat16
    fp8 = mybir.dt.float8e4
    add = mybir.AluOpType.add
    dr = mybir.MatmulPerfMode.DoubleRow

    # time-major layout: segs[p, m*KCH*128 + k*128 + s]; any chunk group is a
    # contiguous free-dim slice, so one DMA per group regardless of size.
    segs_d = nc.dram_tensor("segs", [128, MCH * KCH, 128], fp8, kind="ExternalInput").ap()
    w_d = nc.dram_tensor("w", [128, KCH, NW], fp8, kind="ExternalInput").ap()
    out_d = nc.dram_tensor("out", [128, MCH * NF], bf16, kind="ExternalOutput").ap()

    es = ExitStack()
    seg_sb = {}
    grp_of = {}
    for gi, grp in enumerate(GROUPS):
        t = es.enter_context(
            nc.sbuf_tensor(f"seg{gi}", [128, len(grp) * KCH, 128], fp8)
        )
        for gg, m in enumerate(grp):
            seg_sb[m] = (t, gg)
            grp_of[m] = gi
    w_sb = es.enter_context(nc.sbuf_tensor("wb", [128, KCH, NW], fp8))
    sq_sb = [
        es.enter_context(nc.sbuf_tensor(f"sq{m}", [128, NW], f32))
        for m in range(MCH)
    ]
    out_sb = es.enter_context(nc.sbuf_tensor("ob", [128, MCH * NF], bf16))
    ps = [
        es.enter_context(nc.psum_tensor(f"ps{m}", [128, NW], f32))
        for m in range(MCH)
    ]

    s_grp = [nc.alloc_semaphore(f"s_grp{gi}") for gi in range(len(GROUPS))]
    s_w = nc.alloc_semaphore("s_w")
    s_mm = nc.alloc_semaphore("s_mm")
    s_sq = nc.alloc_semaphore("s_sq")
    s_fold = nc.alloc_semaphore("s_fold")
    s_out = nc.alloc_semaphore("s_out")
    all_sems = [*s_grp, s_w, s_mm, s_sq, s_fold, s_out]

    # input DMAs: vector/gpsimd sequencers come out of the preamble ~1us
    # before sync, so they carry the critical-path c0/w descriptors; every
    # group's descriptor is issued up front so the whole stream lands early.
    def seg_dma(eng, gi):
        grp = GROUPS[gi]
        g0, glen = grp[0], len(grp)
        t = seg_sb[grp[0]][0]
        eng.dma_start(
            t.ap()[:], segs_d[:, g0 * KCH:(g0 + glen) * KCH, :]
        ).then_inc(s_grp[gi], 16)

    nc.scalar.dma_start(w_sb.ap()[:], w_d[:]).then_inc(s_w, 16)
    seg_dma(nc.scalar, 0)                            # c0
    seg_dma(nc.scalar, 1)                            # c1
    seg_dma(nc.sync, 2)                              # c23
    seg_dma(nc.sync, 3)                              # c45
    seg_dma(nc.gpsimd, 4)                            # c67 (SWDGE, least urgent)

    # PE: per chunk, 4 DoubleRow matmuls into the chunk's own PSUM bank
    nc.tensor.wait_ge(s_w, 16)
    seen = set()
    for m in range(MCH):
        gi = grp_of[m]
        if gi not in seen:
            seen.add(gi)
            nc.tensor.wait_ge(s_grp[gi], 16)
        t, gg = seg_sb[m]
        for j in range(KCH // 2):
            inst = nc.tensor.matmul(
                ps[m].ap()[:],
                t.ap()[:, gg * KCH + 2 * j:gg * KCH + 2 * j + 2, :],
                w_sb.ap()[:, 2 * j:2 * j + 2, :],
                start=(j == 0),
                stop=(j == KCH // 2 - 1),
                perf_mode=dr,
            )
        inst.then_inc(s_mm)

    # ACT: squares straight out of PSUM (single-src op, 1 rd port suffices)
    for m in range(MCH):
        nc.scalar.wait_ge(s_mm, m + 1)
        nc.scalar.square(sq_sb[m].ap()[:], ps[m].ap()[:]).then_inc(s_sq)

    # Pool: fold cos^2 + sin^2 into the bf16 out tile
    for m in range(MCH):
        nc.gpsimd.wait_ge(s_sq, m + 1)
        nc.gpsimd.tensor_tensor(
            out=out_sb.ap()[:, m * NF:(m + 1) * NF],
            in0=sq_sb[m].ap()[:, :NF],
            in1=sq_sb[m].ap()[:, NF:],
            op=add,
        ).then_inc(s_fold)

    # writeback in two halves so the first half's descriptor-gen and launch
    # hide under the remaining chunks' compute; halves go on idle engines.
    HF = MCH // 2 * NF
    nc.sync.wait_ge(s_fold, MCH // 2)
    nc.sync.dma_start(out_d[:, :HF], out_sb.ap()[:, :HF]).then_inc(s_out, 16)
    nc.scalar.wait_ge(s_fold, MCH)
    nc.scalar.dma_start(out_d[:, HF:], out_sb.ap()[:, HF:]).then_inc(s_out, 16)
    # single barrier gated on both halves; the compiler's own end-of-program
    # event-sem range clears (which cover our sems too) run after it.
    nc.sync.wait_ge(s_out, 32)
    nc.all_engine_barrier()
    del all_sems
    es.close()

    nc.compile()
    return nc


def _get_nc():
    global _NC
    if _NC is None:
        _NC = _build_nc()
    return _NC


def _prep_in_maps(model_output, offsets):
    model_output = np.ascontiguousarray(model_output, dtype=np.float32)
    off = np.asarray(offsets, dtype=np.int64)
    sw = np.lib.stride_tricks.sliding_window_view(model_output, DT, axis=-1)
    bi = np.arange(B)[:, None, None]
    ci = np.arange(C)[None, :, None]
    seg = sw[bi, ci, off]                       # [B, C, K, DT] f32
    seg8 = seg.astype(ml_dtypes.float8_e4m3fn)
    in_maps = []
    for c in range(NCORES):
        sl = seg8[:, c * CLOC:(c + 1) * CLOC].reshape(SEGS, DT)
        # [p, m, k, s] = seg^T(time k*128+p, window m*128+s), time-major
        arr = np.ascontiguousarray(
            sl.reshape(MCH, 128, KCH, 128).transpose(3, 0, 2, 1)
            .reshape(128, MCH * KCH, 128)
        )
        in_maps.append({"segs": arr, "w": _W_F8})
    return in_maps


def _finish(results):
    s = np.zeros((B, NF), dtype=np.float64)
    sq = np.zeros(B, dtype=np.float64)
    for c in range(NCORES):
        band8 = (
            results[c]["out"].astype(np.float64)
            .reshape(128, MCH, NF).transpose(1, 0, 2)   # [MCH, 128, NF]
        )
        rs = band8.sum(axis=-1)                         # [MCH, 128]
        q = (band8 * band8).sum(axis=-1)                # [MCH, 128]
        pn_sum = band8 / rs[..., None]
        for m in range(MCH):
            v = m // (MCH // B)
            s[v] += pn_sum[m].sum(axis=0)
            sq[v] += (q[m] / (rs[m] * rs[m])).sum()
    n = float(N_TOT)
    pos_per = (2.0 * n * sq - 2.0 * (s * s).sum(-1)) / NF / (n * n - n)
    pos = (pos_per[0] + pos_per[1]) / 2.0
    neg = -(n * sq[0] + n * sq[1] - 2.0 * float(np.dot(s[0], s[1]))) / NF / (n * n)
    return np.float32(pos + neg), np.float32(pos), np.float32(neg)


def kernel(model_output, offsets):
    from concourse.bass_utils import run_bass_kernel_spmd

    nc = _get_nc()
    in_maps = _prep_in_maps(model_output, offsets)
    res = run_bass_kernel_spmd(nc, in_maps, core_ids=list(range(NCORES)))
    return _finish(res.results)


# revision 14
# speedup vs baseline: 1.1390x; 1.1390x over previous
"""Trainium2 Bass kernel for nn_ContrastLoss (band-limited PSD contrastive loss).

Math notes (all exact identities, not approximations):
  - reference subtracts the per-window mean, but integer-frequency DFT bins
    23..102 are orthogonal to DC, so mean subtraction is a no-op on the band.
  - the band PSD is normalized per window (band / band.sum()), so the
    reference's 1/DELTA_T rfft scaling cancels; raw |X_f|^2 suffices.
  - rfft band bins are two real matmuls: X_f = seg @ cos_f, seg @ sin_f.

Device work per core (8-way channel sharding, 1024 windows/core):
  Inputs ship as fp8 e4m3 (validated: scale-rel err ~9e-5 vs 2e-2 gate).
  seg stream = 1MB in 5 DMAs (c0, c1 single chunks for a fast start, 2-chunk
  groups after) across both DGE rings; basis w = 160KB.  Per 128-window
  chunk: 4 DoubleRow fp8 matmuls (contraction 256 each) -> its own PSUM bank
  [128,160] f32; ACT squares PSUM->SBUF; Pool folds cos^2+sin^2 -> [128,80]
  bf16 slices of one out tile; one 160KB writeback.

Synchronization is hand-rolled (no TileContext): 11 semaphores, chunk-level
producer counters per engine pair.  The TileContext teardown (one sem-clear
instruction per allocated sem per engine, ~50 sems x ~115ns) was ~8us of the
baseline exec time; here the epilogue is two barriers + one range-clear.
Host: window gather + fp8 shard prep; row-sums, normalization and the
closed-form pairwise-MSE scalars in float64 (cheap: 8x1024x80 values).
"""

import sys

import numpy as np

if "/opt/trn_rl_repo" not in sys.path:
    sys.path.insert(0, "/opt/trn_rl_repo")

import ml_dtypes

B = 2
C = 256
T = 8192
K = 16
DT = 1024
NCORES = 8
CLOC = C // NCORES          # channels per core
SEGS = B * CLOC * K         # windows per core = 1024
F_LO, F_HI = 23, 103        # band bins [23, 102]
NF = F_HI - F_LO            # 80
NW = 2 * NF                 # 160 (cos || sin)
MCH = SEGS // 128           # 8 window chunks
KCH = DT // 128             # 8 contraction chunks
N_TOT = C * K               # 4096 windows per video
GROUPS = ((0,), (1,), (2, 3), (4, 5), (6, 7))   # seg DMA batching


def _dft_basis():
    t = np.arange(DT, dtype=np.float64)
    f = np.arange(F_LO, F_HI, dtype=np.float64)
    ang = 2.0 * np.pi * np.outer(t, f) / DT
    w = np.concatenate([np.cos(ang), np.sin(ang)], axis=1).astype(np.float32)
    w8 = w.astype(ml_dtypes.float8_e4m3fn)      # [DT, NW]
    # [p, k, j] = W[k*128 + p, j] so rhs for k-pair is a dim-1 slice
    return np.ascontiguousarray(
        w8.reshape(KCH, 128, NW).transpose(1, 0, 2)
    )


_W_F8 = _dft_basis()
_NC = None


def _build_nc():
    from contextlib import ExitStack

    import concourse.mybir as mybir
    from concourse import bacc

    nc = bacc.Bacc(
        "TRN2",
        target_bir_lowering=False,
        debug=False,
        enable_asserts=True,
        num_devices=NCORES,
    )
    f32 = mybir.dt.float32
    bf16 = mybir.dt.bfloat16
    fp8 = mybir.dt.float8e4
    add = mybir.AluOpType.add
    dr = mybir.MatmulPerfMode.DoubleRow

    # time-major layout: segs[p, m*KCH*128 + k*128 + s]; any chunk group is a
    # contiguous free-dim slice, so one DMA per group regardless of size.
    segs_d = nc.dram_tensor("segs", [128, MCH * KCH, 128], fp8, kind="ExternalInput").ap()
    w_d = nc.dram_tensor("w", [128, KCH, NW], fp8, kind="ExternalInput").ap()
    out_d = nc.dram_tensor("out", [128, MCH * NF], bf16, kind="ExternalOutput").ap()

    es = ExitStack()
    seg_sb = {}
    grp_of = {}
    for gi, grp in enumerate(GROUPS):
        t = es.enter_context(
            nc.sbuf_tensor(f"seg{gi}", [128, len(grp) * KCH, 128], fp8)
        )
        for gg, m in enumerate(grp):
            seg_sb[m] = (t, gg)
            grp_of[m] = gi
    w_sb = es.enter_context(nc.sbuf_tensor("wb", [128, KCH, NW], fp8))
    sq_sb = [
        es.enter_context(nc.sbuf_tensor(f"sq{m}", [128, NW], f32))
        for m in range(MCH)
    ]
    out_sb = es.enter_context(nc.sbuf_tensor("ob", [128, MCH * NF], bf16))
    ps = [
        es.enter_context(nc.psum_tensor(f"ps{m}", [128, NW], f32))
        for m in range(MCH)
    ]

    s_grp = [nc.alloc_semaphore(f"s_grp{gi}") for gi in range(len(GROUPS))]
    s_w = nc.alloc_semaphore("s_w")
    s_mm = nc.alloc_semaphore("s_mm")
    s_sq = nc.alloc_semaphore("s_sq")
    s_fold = nc.alloc_semaphore("s_fold")
    s_out = nc.alloc_semaphore("s_out")
    all_sems = [*s_grp, s_w, s_mm, s_sq, s_fold, s_out]

    # input DMAs: vector/gpsimd sequencers come out of the preamble ~1us
    # before sync, so they carry the critical-path c0/w descriptors; every
    # group's descriptor is issued up front so the whole stream lands early.
    def seg_dma(eng, gi):
        grp = GROUPS[gi]
        g0, glen = grp[0], len(grp)
        t = seg_sb[grp[0]][0]
        eng.dma_start(
            t.ap()[:], segs_d[:, g0 * KCH:(g0 + glen) * KCH, :]
        ).then_inc(s_grp[gi], 16)

    seg_dma(nc.sync, 0)                              # c0      first on ring A
    nc.scalar.dma_start(w_sb.ap()[:], w_d[:]).then_inc(s_w, 16)  # w first on B
    seg_dma(nc.sync, 1)                              # c1
    seg_dma(nc.scalar, 2)                            # c23
    seg_dma(nc.sync, 3)                              # c45
    seg_dma(nc.scalar, 4)                            # c67

    # PE: per chunk, 4 DoubleRow matmuls into the chunk's own PSUM bank
    nc.tensor.wait_ge(s_w, 16)
    seen = set()
    for m in range(MCH):
        gi = grp_of[m]
        if gi not in seen:
            seen.add(gi)
            nc.tensor.wait_ge(s_grp[gi], 16)
        t, gg = seg_sb[m]
        for j in range(KCH // 2):
            inst = nc.tensor.matmul(
                ps[m].ap()[:],
                t.ap()[:, gg * KCH + 2 * j:gg * KCH + 2 * j + 2, :],
                w_sb.ap()[:, 2 * j:2 * j + 2, :],
                start=(j == 0),
                stop=(j == KCH // 2 - 1),
                perf_mode=dr,
            )
        inst.then_inc(s_mm)

    # ACT: squares straight out of PSUM (single-src op, 1 rd port suffices)
    for m in range(MCH):
        nc.scalar.wait_ge(s_mm, m + 1)
        nc.scalar.square(sq_sb[m].ap()[:], ps[m].ap()[:]).then_inc(s_sq)

    # Pool: fold cos^2 + sin^2 into the bf16 out tile
    for m in range(MCH):
        nc.gpsimd.wait_ge(s_sq, m + 1)
        nc.gpsimd.tensor_tensor(
            out=out_sb.ap()[:, m * NF:(m + 1) * NF],
            in0=sq_sb[m].ap()[:, :NF],
            in1=sq_sb[m].ap()[:, NF:],
            op=add,
        ).then_inc(s_fold)

    # writeback in two halves so the first half's descriptor-gen and launch
    # hide under the remaining chunks' compute; halves go on idle engines.
    HF = MCH // 2 * NF
    nc.sync.wait_ge(s_fold, MCH // 2)
    nc.sync.dma_start(out_d[:, :HF], out_sb.ap()[:, :HF]).then_inc(s_out, 16)
    nc.scalar.wait_ge(s_fold, MCH)
    nc.scalar.dma_start(out_d[:, HF:], out_sb.ap()[:, HF:]).then_inc(s_out, 16)
    # single barrier gated on both halves; the compiler's own end-of-program
    # event-sem range clears (which cover our sems too) run after it.
    nc.sync.wait_ge(s_out, 32)
    nc.all_engine_barrier()
    del all_sems
    es.close()

    nc.compile()
    return nc


def _get_nc():
    global _NC
    if _NC is None:
        _NC = _build_nc()
    return _NC


def _prep_in_maps(model_output, offsets):
    model_output = np.ascontiguousarray(model_output, dtype=np.float32)
    off = np.asarray(offsets, dtype=np.int64)
    sw = np.lib.stride_tricks.sliding_window_view(model_output, DT, axis=-1)
    bi = np.arange(B)[:, None, None]
    ci = np.arange(C)[None, :, None]
    seg = sw[bi, ci, off]                       # [B, C, K, DT] f32
    seg8 = seg.astype(ml_dtypes.float8_e4m3fn)
    in_maps = []
    for c in range(NCORES):
        sl = seg8[:, c * CLOC:(c + 1) * CLOC].reshape(SEGS, DT)
        # [p, m, k, s] = seg^T(time k*128+p, window m*128+s), time-major
        arr = np.ascontiguousarray(
            sl.reshape(MCH, 128, KCH, 128).transpose(3, 0, 2, 1)
            .reshape(128, MCH * KCH, 128)
        )
        in_maps.append({"segs": arr, "w": _W_F8})
    return in_maps


def _finish(results):
    s = np.zeros((B, NF), dtype=np.float64)
    sq = np.zeros(B, dtype=np.float64)
    for c in range(NCORES):
        band8 = (
            results[c]["out"].astype(np.float64)
            .reshape(128, MCH, NF).transpose(1, 0, 2)   # [MCH, 128, NF]
        )
        rs = band8.sum(axis=-1)                         # [MCH, 128]
        q = (band8 * band8).sum(axis=-1)                # [MCH, 128]
        pn_sum = band8 / rs[..., None]
        for m in range(MCH):
            v = m // (MCH // B)
            s[v] += pn_sum[m].sum(axis=0)
            sq[v] += (q[m] / (rs[m] * rs[m])).sum()
    n = float(N_TOT)
    pos_per = (2.0 * n * sq - 2.0 * (s * s).sum(-1)) / NF / (n * n - n)
    pos = (pos_per[0] + pos_per[1]) / 2.0
    neg = -(n * sq[0] + n * sq[1] - 2.0 * float(np.dot(s[0], s[1]))) / NF / (n * n)
    return np.float32(pos + neg), np.float32(pos), np.float32(neg)


def kernel(model_output, offsets):
    from concourse.bass_utils import run_bass_kernel_spmd

    nc = _get_nc()
    in_maps = _prep_in_maps(model_output, offsets)
    res = run_bass_kernel_spmd(nc, in_maps, core_ids=list(range(NCORES)))
    return _finish(res.results)
